# revision 51
# baseline (speedup 1.0000x reference)
"""Trainium2 Bass kernel for nn_Bottleneck_SAA (CSP bottleneck + dual PAM attention).

Sharding: 8 cores = 4 batches x 2 row-halves. One SPMD program; odd cores
receive a vertically flipped image + vertically flipped conv kernels, so
every core computes output rows 0..31 of its (possibly flipped) input
(conv(flip(x), flip_h(w)) == flip(conv(x, w)); attention is invariant to
permuting the softmax axis). The host flips those outputs back.

Per-core on-chip pipeline (fp16 matmul operands, fp32 PSUM accumulate):
  conv1(3x3, BN+SiLU folded into weights/bias) -> conv2 -> q/k/vT
  projections -> flash-style attention in [m, n] orientation:
    energyT = k_chunk^T q  (PSUM) -> exp (ACT, fp8 out) ->
    outT[65, n] += [vT | 1]^T expT   (row 64 = softmax denominator)
  normalization via DVE reciprocal + K=1 ones-matmul partition broadcast;
  residual fused: out = x + 2*y + 2*gamma*(attn_out + v_b).  2*gamma is
  folded into vwT host-side; 2*gamma*v_b rides a bias row appended to the
  v projection (the softmax denominator cancels it exactly), so the
  residual input x is re-read straight out of the xs conv stack -- no
  separate xh input tensor.

v3 scheduling changes (141.9us baseline):
  - HAM warm-up: the PE clock-gates to 1.2 GHz unless the ARRAY (not the
    instruction stream -- v2 measured 100%-issue-busy K=8 matmuls staying
    cold for 106us) is kept active.  The K=8 energy matmuls light up only
    8/128 rows, so they never register as busy.  Fix: zero-pad the
    energy contraction to K=128 (k_ext/q_ext rows 8-127 are zeros --
    exact, and the 512-cycle stream cost is K-independent).  The conv
    phase (K=128/96) already holds K=8/8 warm (v2: 18.5->47us).
  - explicit keep-warm dummy matmuls are a NET LOSS: interleaving
    different array configs (col_grp/row_grp/perf-mode switches) defeats
    the drain/fill overlap between consecutive matmuls (+250ns/MM,
    measured v2) and they still don't register occupancy.  Don't.
  - startup: the first conv matmul is HBM-bandwidth-bound, so the xs
    stack ships only the x half (557KB); the [x<<1] copy is made
    on-chip by 4 chunked SBUF->SBUF shift DMAs.  Weights packed into two
    DMA blobs; xs split into 4 column chunks across the
    sync/scalar/gpsimd queues; conv1's first group shrunk to 2 tiles so
    it only waits for xs cols 0:1056.
  - final out DMAs for the last half split across queues (the ~2us
    HBM-write receipt latency parallelizes).

Conv tricks: every pass streams ONE contiguous span of the zero-padded
[66x66] image (row-tap start offsets keep passes column-aligned in
PSUM; pad-column garbage is skipped at SiLU evacuation). Column taps
(u,0)/(u,1) ride the CONTRACTION axis via the host-built [x, x<<1]
stack; conv2 stacks [y1, y1<<1, y1<<2] on 96 partitions so all 3 column
taps fuse into K=96 (3 passes instead of 9). conv1 reads ONLY the
[x, x<<1] stack: taps (u,2) come from streaming the x block at a +2
column offset (3x K=128 + 3x K=64 passes).
"""

import sys

sys.path.insert(0, "/opt/trn_rl_repo")

from contextlib import ExitStack

import numpy as np
import ml_dtypes

import concourse.bass as bass
import concourse.tile as tile
from concourse import bacc, mybir
from concourse.bass_utils import run_bass_kernel_spmd

B, C1, C2, Cm, C8 = 4, 64, 64, 32, 8
H = W = 64
N = H * W            # 4096 pixels
NH = N // 2          # 2048 pixels per core (32 rows)
HP = H + 2           # padded height
WP = W + 2
NP = HP * WP         # 4356
NCORES = 8
EPS = 1e-5
FP32 = mybir.dt.float32
AF = mybir.ActivationFunctionType
ALU = mybir.AluOpType

MCHUNKS = N // 128   # 32 attention m-chunks
NSPAN = 1024         # n columns processed per accumulator half
BF16 = mybir.dt.float16  # 16-bit matmul operand dtype (fp16: 8x less rounding than bf16)
FP8 = mybir.dt.float8e4   # e4m3 for the attention-weights matmul (DoubleRow)
VP = 80                   # padded per-chunk lhsT columns (65 -> 80, step%16==0)
RPT = 7              # conv: image rows per matmul (contiguous-stream tiling)

# xs column chunk boundaries (one DMA per chunk, one queue each)
XCH = [0, 16 * WP, 32 * WP, 49 * WP, NP]

_build_cache = {}


def _build_program():
    if "nc" in _build_cache:
        return _build_cache["nc"]
    nc = bacc.Bacc("TRN2", target_bir_lowering=False, debug=False, num_devices=NCORES)

    xp_d = nc.dram_tensor("xs", [C1, NP], BF16, kind="ExternalInput")
    w1_d = nc.dram_tensor("w1pack", [128, 3 * 3 * Cm], BF16, kind="ExternalInput")
    b1_d = nc.dram_tensor("b1", [Cm, 1], FP32, kind="ExternalInput")
    w2_d = nc.dram_tensor("w2pack", [96, 3 * C2 + 2 * C8 + C2], BF16, kind="ExternalInput")
    b2_d = nc.dram_tensor("b2", [C2, 1], FP32, kind="ExternalInput")
    out_d = nc.dram_tensor("out", [C2, NH], FP32, kind="ExternalOutput")

    with tile.TileContext(nc) as tc:
        with ExitStack() as ctx:
            per = ctx.enter_context(tc.tile_pool(name="persist", bufs=1))

            xs_sb = per.tile([128, NP], BF16)
            w1p_sb = per.tile([128, 3 * 3 * Cm], BF16)
            b1_sb = per.tile([Cm, 1], FP32)
            w2p_sb = per.tile([96, 3 * C2 + 2 * C8 + C2], BF16)
            b2_sb = per.tile([C2, 1], FP32)
            ones_sb = per.tile([1, C2], BF16)

            w1_sb = w1p_sb[:, 0:3 * Cm]
            w1b_sb = w1p_sb[0:C1, 3 * Cm:6 * Cm]
            w1c_sb = w1p_sb[0:C1, 6 * Cm:9 * Cm]   # tap (u,1) at partition base 0
            w2_sb = w2p_sb[0:96, 0:3 * C2]
            qw_sb = w2p_sb[0:C2 + 1, 3 * C2:3 * C2 + C8]
            kw_sb = w2p_sb[0:C2 + 1, 3 * C2 + C8:3 * C2 + 2 * C8]
            vw_sb = w2p_sb[0:C2 + 1, 3 * C2 + 2 * C8:3 * C2 + 2 * C8 + C2]

            ys_sb = per.tile([96, NP], BF16)       # conv1 out + 2 column-shifted copies
            y_sb = per.tile([C2 + 1, N], BF16)     # conv2 output, row 64 = ones (bias)
            k_sb = per.tile([128, N], BF16)        # rows 0-7 = k, rows 8-127 = 0
            q_sb = per.tile([128, NH], BF16)       # rows 0-7 = q, rows 8-127 = 0
            vext_sb = per.tile([128, (MCHUNKS // 2) * 2 * VP], FP8)  # [128, 16, 2, 80]
            r_sb = per.tile([C2, NH], FP32)        # x_half + 2*y_half
            fin_sb = per.tile([C2, NH], FP32)

            # startup DMA plan: the conv1 critical set (w1pack + xs chunk 0)
            # leads two different queues; the [x<<1] stack half is built
            # on-chip by chunked SBUF->SBUF shift DMAs (x<<1 from HBM would
            # double the startup HBM traffic).  A shift trigger WAITS on its
            # source chunk's semaphore and blocks everything behind it on
            # its engine queue, so shifts are issued LAST per queue.  conv1
            # group 0 uses the 9-pass x-only form, so no shift gates it.
            nc.sync.dma_start(xs_sb[0:C1, XCH[0]:XCH[1]], xp_d.ap()[:, XCH[0]:XCH[1]])
            nc.scalar.dma_start(w1p_sb[:], w1_d.ap())
            nc.gpsimd.dma_start(xs_sb[0:C1, XCH[2]:XCH[3]], xp_d.ap()[:, XCH[2]:XCH[3]])
            nc.scalar.dma_start(xs_sb[0:C1, XCH[1]:XCH[2]], xp_d.ap()[:, XCH[1]:XCH[2]])
            nc.sync.dma_start(xs_sb[0:C1, XCH[3]:XCH[4]], xp_d.ap()[:, XCH[3]:XCH[4]])
            nc.sync.dma_start(w2p_sb[:], w2_d.ap())
            nc.scalar.dma_start(b1_sb[:], b1_d.ap())
            nc.scalar.dma_start(b2_sb[:], b2_d.ap())
            # shift DMAs (dest chunk c reads source cols +1, all within the
            # same source chunk; boundaries offset by one column)
            nc.sync.dma_start(xs_sb[C1:128, XCH[0]:XCH[1] - 1],
                              xs_sb[0:C1, XCH[0] + 1:XCH[1]])
            nc.scalar.dma_start(xs_sb[C1:128, XCH[1] - 1:XCH[2] - 1],
                                xs_sb[0:C1, XCH[1]:XCH[2]])
            nc.gpsimd.dma_start(xs_sb[C1:128, XCH[2] - 1:XCH[3] - 1],
                                xs_sb[0:C1, XCH[2]:XCH[3]])
            nc.sync.dma_start(xs_sb[C1:128, XCH[3] - 1:XCH[4] - 1],
                              xs_sb[0:C1, XCH[3]:XCH[4]])

            # zero the padded contraction rows of q/k ONCE (both sides:
            # 0 * NaN-garbage would still poison the energy sums).  Whole
            # tile -- engine APs need 32-aligned partition bases; the
            # projections overwrite rows 0-7 later.
            nc.vector.memset(q_sb[:], 0.0)
            nc.vector.memset(k_sb[:], 0.0)

            nc.gpsimd.memset(ones_sb[:], 1.0)
            nc.gpsimd.memset(y_sb[C2:C2 + 1, :], 1.0)

            ys_v = ys_sb[:].rearrange("p (a b) -> p a b", b=WP)
            # zero only what conv2 actually reads as padding: rows 0/65
            # everywhere, pad cols 0/65 on the unshifted block (the shifted
            # blocks inherit correct pads from the chunked DMAs).
            nc.gpsimd.memset(ys_sb[:, 0:WP], 0.0)
            nc.gpsimd.memset(ys_sb[:, (HP - 1) * WP:NP], 0.0)
            nc.gpsimd.memset(ys_v[0:Cm, 1:HP - 1, 0:1], 0.0)
            nc.gpsimd.memset(ys_v[0:Cm, 1:HP - 1, WP - 1:WP], 0.0)

            vext_v = vext_sb[:].rearrange("p (c s k) -> p c s k", s=2, k=VP)
            nc.gpsimd.memset(vext_v[:, :, :, C2:], 0.0)
            nc.gpsimd.memset(vext_v[:, :, :, C2:C2 + 1], 1.0)

            xs_v = xs_sb[:].rearrange("p (a b) -> p a b", b=WP)
            y_v = y_sb[0:C2, :]
            y_rows = y_sb[0:C2, :].rearrange("p (a b) -> p a b", b=W)
            r_rows = r_sb[:].rearrange("p (a b) -> p a b", b=W)
            y65 = y_sb[:]

            # conv tiling: groups of RPT image rows; each tap streams one
            # CONTIGUOUS span of the padded image (garbage at the 2 pad
            # columns per row accumulates in psum and is skipped on
            # evacuation).
            conv_tiles = [(RPT * t, RPT) for t in range(H // RPT)]
            if H % RPT:
                conv_tiles.append((H - H % RPT, H % RPT))

            # conv1: 6 streamed passes/tile (the contraction-lower-bound):
            # 3x K=128 on xs=[x, x<<1] (taps (u,0)+(u,1)), 3x K=64 on the x
            # block at a +2 column offset (taps (u,2)).  Group 0 instead
            # runs 9x K=64 on the x block alone (taps (u,c) at +c column
            # offsets) so it isn't gated on the on-chip x<<1 shift DMA.
            def conv1_group(psA, tiles, nine=False):
                ps = psA.tile([128, WP * RPT], FP32, tag="mm")
                def mm(g, pass_i):
                    r0, nr = tiles[g]
                    length = WP * (nr - 1) + W
                    pslc = ps[Cm * g:Cm * (g + 1), 0:length]
                    tp = (0, Cm * g)
                    if nine:
                        u, c = pass_i // 3, pass_i % 3
                        lhs = (w1_sb[0:C1, Cm * u:Cm * (u + 1)] if c == 0 else
                               w1c_sb[:, Cm * u:Cm * (u + 1)] if c == 1 else
                               w1b_sb[:, Cm * u:Cm * (u + 1)])
                        s = (r0 + u) * WP + c
                        nc.tensor.matmul(
                            pslc, lhs, xs_sb[0:C1, s:s + length],
                            start=(pass_i == 0), stop=(pass_i == 8),
                            tile_position=tp,
                        )
                    elif pass_i < 3:
                        s = (r0 + pass_i) * WP
                        nc.tensor.matmul(
                            pslc, w1_sb[:, Cm * pass_i:Cm * (pass_i + 1)],
                            xs_sb[:, s:s + length],
                            start=(pass_i == 0), stop=False, tile_position=tp,
                        )
                    else:
                        u = pass_i - 3
                        s = (r0 + u) * WP + 2
                        nc.tensor.matmul(
                            pslc, w1b_sb[:, Cm * u:Cm * (u + 1)],
                            xs_sb[0:C1, s:s + length],
                            start=False, stop=(pass_i == 5), tile_position=tp,
                        )
                for pass_i in range(9 if nine else 6):
                    for g in range(len(tiles)):
                        mm(g, pass_i)
                ps_v = ps[:].rearrange("p (r w) -> p r w", w=WP)
                for g, (r0, nr) in enumerate(tiles):
                    silu2(ys_v[0:Cm, 1 + r0:1 + r0 + nr, 1:1 + W],
                          ps_v[Cm * g:Cm * (g + 1), 0:nr, 0:W], b1_sb, Cm, nr)

            # conv2: all 3 column taps on the partition axis (K=96, shifted
            # copies of y1 at rows 32-63 / 64-95): 3 passes instead of 9.
            def conv2_group(psA, tiles):
                ps = psA.tile([128, WP * RPT], FP32, tag="mm")
                for u in range(3):
                    for g, (r0, nr) in enumerate(tiles):
                        length = WP * (nr - 1) + W
                        s = (r0 + u) * WP
                        nc.tensor.matmul(
                            ps[C2 * g:C2 * (g + 1), 0:length],
                            w2_sb[:, C2 * u:C2 * (u + 1)],
                            ys_sb[:, s:s + length], start=(u == 0), stop=(u == 2),
                            tile_position=(0, C2 * g),
                        )
                ps_v = ps[:].rearrange("p (r w) -> p r w", w=WP)
                for g, (r0, nr) in enumerate(tiles):
                    silu2(y_rows[:, r0:r0 + nr, :],
                          ps_v[C2 * g:C2 * (g + 1), 0:nr, 0:W], b2_sb, C2, nr)

            with (
                tc.tile_pool(name="psA", bufs=6, space="PSUM") as psA,
                tc.tile_pool(name="sact", bufs=6) as sact,
            ):
                def silu2(dst, psv, b_sb, cp, nr):
                    # silu(z) = (z/2)*(1+tanh(z/2)), z = P+b: one ACT tanh +
                    # two DVE ops.  Tanh lives in the SAME ACT table set as
                    # exp, so no table reload sits between the conv phase
                    # and the attention exp stream (a SILU set switch costs
                    # 1.28us right before the first exp).  b_sb holds 0.5*b.
                    tt = sact.tile([C2, RPT * W], BF16, tag="tt")
                    zz = sact.tile([C2, RPT * W], BF16, tag="zz")
                    tv = tt[:].rearrange("p (r w) -> p r w", w=W)[0:cp, 0:nr, :]
                    zv = zz[:].rearrange("p (r w) -> p r w", w=W)[0:cp, 0:nr, :]
                    nc.scalar.activation(tv, psv, AF.Tanh,
                                         bias=b_sb[:, 0:1], scale=0.5)
                    nc.vector.tensor_scalar(zv, psv, 0.5, b_sb[:, 0:1],
                                            ALU.mult, ALU.add)
                    nc.vector.scalar_tensor_tensor(dst, tv, 1.0, zv,
                                                   ALU.add, ALU.mult)

                # conv1 with the y1 column-shift DMAs chunked in right behind
                # the tiles that produce their source rows.  First group is 2
                # tiles so it starts as soon as xs chunk 0 (cols 0:1056) lands.
                c1groups = [conv_tiles[0:2], conv_tiles[2:6], conv_tiles[6:10]]
                for gi, grp in enumerate(c1groups):
                    conv1_group(psA, grp, nine=(gi == 0))
                    for c0 in range(0, len(grp), 2):
                        sub = grp[c0:c0 + 2]
                        a = (1 + sub[0][0]) * WP
                        b = (1 + sub[-1][0] + sub[-1][1]) * WP
                        nc.sync.dma_start(ys_sb[Cm:2 * Cm, a - 1:b - 1],
                                          ys_sb[0:Cm, a:b])
                        nc.gpsimd.dma_start(ys_sb[2 * Cm:3 * Cm, a - 2:b - 2],
                                            ys_sb[0:Cm, a:b])

                # conv2 with projections (and the residual) interleaved; a
                # projection issues one tile AFTER its y span completes so the
                # in-order PE never parks on a SiLU that hasn't drained yet.
                k_done = q_done = vg_done = r_done = 0
                groups = [conv_tiles[i:i + 2] for i in range(0, len(conv_tiles), 2)]
                for grp in groups + [None]:
                    if grp is not None:
                        conv2_group(psA, grp)
                    rows = grp[0][0] if grp is not None else H
                    while q_done < min(4, rows // 8):
                        t0 = q_done
                        ps = psA.tile([C8, 512], FP32, tag="mm")
                        nc.tensor.matmul(ps[:], qw_sb[:],
                                         y65[:, 512 * t0:512 * (t0 + 1)],
                                         start=True, stop=True)
                        nc.vector.tensor_copy(q_sb[0:C8, 512 * t0:512 * (t0 + 1)],
                                              ps[:])
                        q_done += 1
                    while k_done < min(8, rows // 8):
                        t0 = k_done
                        ps = psA.tile([C8, 512], FP32, tag="mm")
                        nc.tensor.matmul(ps[:], kw_sb[:],
                                         y65[:, 512 * t0:512 * (t0 + 1)],
                                         start=True, stop=True)
                        nc.vector.tensor_copy(k_sb[0:C8, 512 * t0:512 * (t0 + 1)],
                                              ps[:])
                        k_done += 1
                    while vg_done < rows // 16:
                        g = vg_done
                        ps = psA.tile([128, 512], FP32, tag="mm")
                        for i in range(8):
                            j = 8 * g + i
                            nc.tensor.matmul(
                                ps[:, C2 * i:C2 * (i + 1)],
                                y65[:, 128 * j:128 * (j + 1)],
                                vw_sb[:],
                                start=True, stop=True,
                            )
                        nc.vector.tensor_copy(vext_v[:, 4 * g:4 * (g + 1), :, 0:C2],
                                              ps[:])
                        vg_done += 1
                    while r_done < min(2, rows // 16):
                        c = r_done
                        rs = slice(16 * c, 16 * (c + 1))
                        nc.vector.scalar_tensor_tensor(
                            r_rows[:, rs, :], y_rows[:, rs, :], 2.0,
                            xs_v[0:C1, 1 + 16 * c:1 + 16 * (c + 1), 1:1 + W],
                            ALU.mult, ALU.add,
                        )
                        r_done += 1

            # ---- attention, two sequential 1024-col halves ----
            with (
                tc.tile_pool(name="psE", bufs=3, space="PSUM") as psE,
                tc.tile_pool(name="psO", bufs=1, space="PSUM") as psO,
                tc.tile_pool(name="expp", bufs=4) as expp,
            ):
                for nh in range(2):
                    cs = NSPAN * nh
                    po = psO.tile([128, NSPAN], FP32, tag="po")
                    pending = []
                    dr_rem = []
                    for p in range(MCHUNKS // 2):
                        exv = expp.tile([128, 2 * NSPAN], FP8, tag="ex")
                        exv = exv[:].rearrange("q (s n) -> q s n", s=2)
                        for s in range(2):
                            i = 2 * p + s
                            pe = psE.tile([128, NSPAN], FP32, tag="pe")
                            for j in range(2):
                                # sandwich each lagged DR matmul between
                                # energy matmuls so its LDWEIGHTS hides
                                # under the neighbour's streaming
                                if dr_rem:
                                    _mm1(nc, po, vext_v, *dr_rem.pop(0))
                                nc.tensor.matmul(
                                    pe[:, 512 * j:512 * (j + 1)],
                                    k_sb[:, 128 * i:128 * (i + 1)],
                                    q_sb[:, cs + 512 * j:cs + 512 * (j + 1)],
                                    start=True, stop=True,
                                )
                            nc.scalar.activation(exv[:, s, :], pe[:], AF.Exp)
                        pending.append((exv, p))
                        if len(pending) > 3:
                            e0, p0 = pending.pop(0)
                            dr_rem += [(e0, p0, 0), (e0, p0, 1)]
                    for t in dr_rem:
                        _mm1(nc, po, vext_v, *t)
                    for e0, p0 in pending:
                        _mm1(nc, po, vext_v, e0, p0, 0)
                        _mm1(nc, po, vext_v, e0, p0, 1)

                    # ---- epilogue for this half, pipelined in 512-col blocks;
                    # all elementwise work on the (idle) DVE so ACT stays on
                    # the exp table for the next half's attention.
                    # 1/sumexp: bounce the [1,1024] denominator row through a
                    # [32,32] tile via DMA so the (per-column-costed) DVE
                    # reciprocal runs over 32 columns (0.36us) instead of 1024.
                    o_sb = per.tile([C2 + 1, NSPAN], FP32, tag="o_sb")
                    rec16 = per.tile([1, NSPAN], BF16, tag="rec16")
                    t1 = per.tile([C2, NSPAN], FP32, tag="t1")
                    zt0 = per.tile([16, 32], FP32, tag="zt0")
                    zr0 = per.tile([16, 32], FP32, tag="zr0")
                    zc0 = per.tile([16, 32], BF16, tag="zc0")
                    zt1 = per.tile([16, 32], FP32, tag="zt1")
                    zr1 = per.tile([16, 32], FP32, tag="zr1")
                    zc1 = per.tile([16, 32], BF16, tag="zc1")
                    zs = [(zt0, zr0, zc0), (zt1, zr1, zc1)]
                    # per-j denominator chains: the two SBUF bounce-DMA
                    # latencies pipeline instead of serializing.  Split
                    # [65,512] evacuations: rows 0-63 = numerator, row 64 =
                    # denominator (a separate [1,1024] copy costs a full
                    # 1024 per-column DVE pass on its own).
                    for j in range(2):
                        sl = slice(512 * j, 512 * (j + 1))
                        nc.vector.tensor_copy(o_sb[:, sl], po[0:C2 + 1, sl])
                        nc.sync.dma_start(zs[j][0][:], o_sb[C2:C2 + 1, sl])
                    for j in range(2):
                        sl = slice(512 * j, 512 * (j + 1))
                        nc.vector.reciprocal(zs[j][1][:], zs[j][0][:])
                        nc.vector.tensor_copy(zs[j][2][:], zs[j][1][:])
                        nc.sync.dma_start(rec16[:, sl], zs[j][2][:])
                    for j in range(2):
                        sl = slice(512 * j, 512 * (j + 1))
                        gl = slice(cs + 512 * j, cs + 512 * (j + 1))
                        nc.tensor.matmul(po[0:C2, sl], ones_sb[:], rec16[:, sl],
                                         start=True, stop=True)
                        nc.vector.tensor_mul(t1[:, sl], o_sb[0:C2, sl], po[0:C2, sl])
                        nc.vector.tensor_add(fin_sb[:, gl], t1[:, sl], r_sb[:, gl])
                        if nh == 0:
                            # sync only: the scalar queue is the ACT engine --
                            # a DMA trigger there would stall half 2's exps.
                            nc.sync.dma_start(out_d.ap()[:, gl], fin_sb[:, gl])
                        else:
                            ga, gb = cs + 512 * j, cs + 512 * j + 256
                            e0, e1 = (nc.sync, nc.gpsimd) if j == 0 else (
                                nc.scalar, nc.gpsimd)
                            e0.dma_start(out_d.ap()[:, ga:gb], fin_sb[:, ga:gb])
                            e1.dma_start(out_d.ap()[:, gb:gb + 256],
                                         fin_sb[:, gb:gb + 256])

    nc.compile()
    _build_cache["nc"] = nc
    return nc


def _mm1(nc, po, vext_v, exv, p, j):
    # DoubleRow fp8: contract 256 m-rows (chunk pair 2p, 2p+1) per pass.
    # po[m, n] += sum_s vext_{2p+s}[:, m]^T expT_{2p+s}[:, n]; row C2 = sum(exp)
    nc.tensor.matmul(
        po[0:VP, 512 * j:512 * (j + 1)],
        vext_v[:, p, :, :],
        exv[:, :, 512 * j:512 * (j + 1)],
        start=(p == 0), stop=(p == MCHUNKS // 2 - 1),
        perf_mode=mybir.MatmulPerfMode.DoubleRow,
    )


def _host_prep(inputs):
    f32 = np.float32
    x = np.asarray(inputs["x"], f32)
    s1 = np.asarray(inputs["bn1_g"], f32) / np.sqrt(np.asarray(inputs["bn1_v"], f32) + EPS)
    bb1 = np.asarray(inputs["bn1_b"], f32) - np.asarray(inputs["bn1_m"], f32) * s1
    w1 = np.asarray(inputs["cv1_w"], f32) * s1[:, None, None, None]
    s2 = np.asarray(inputs["bn2_g"], f32) / np.sqrt(np.asarray(inputs["bn2_v"], f32) + EPS)
    bb2 = np.asarray(inputs["bn2_b"], f32) - np.asarray(inputs["bn2_m"], f32) * s2
    w2 = np.asarray(inputs["cv2_w"], f32) * s2[:, None, None, None]
    gamma = f32(np.asarray(inputs["pam_gamma"], f32))

    bf = np.float16

    def aug(w, b):
        # [Co, C2] weight + [Co] bias -> [C2+1, Co] lhsT with bias row
        co = np.asarray(w, f32).shape[0]
        a = np.zeros((C2 + 1, co), f32)
        a[0:C2, :] = np.asarray(w, f32).T
        a[C2, :] = np.asarray(b, f32)
        return a.astype(bf)

    # w2pack: [96, 3*C2 | qwa(8) | kwa(8) | vwB(64)]
    w2pack = np.zeros((96, 3 * C2 + 2 * C8 + C2), bf)
    w2pack[0:C2 + 1, 3 * C2:3 * C2 + C8] = aug(inputs["q_w"], inputs["q_b"])
    w2pack[0:C2 + 1, 3 * C2 + C8:3 * C2 + 2 * C8] = aug(inputs["k_w"], inputs["k_b"])
    w2pack[0:C2 + 1, 3 * C2 + 2 * C8:] = aug(
        2.0 * gamma * np.asarray(inputs["v_w"], f32),
        2.0 * gamma * np.asarray(inputs["v_b"], f32))

    common = {
        # halved: the kernel computes silu(z) = (z/2)*(1+tanh(z/2)) and
        # builds z/2 as 0.5*psum + (0.5*b)
        "b1": np.ascontiguousarray(0.5 * bb1[:, None]),
        "b2": np.ascontiguousarray(0.5 * bb2[:, None]),
    }

    def packs(w1f, w2f):
        a = np.zeros((128, 9 * Cm), np.float32)
        s2p = np.zeros((96, 3 * C2), np.float32)
        for u in range(3):
            a[0:C1, Cm * u:Cm * (u + 1)] = w1f[:, :, u, 0].T
            a[C1:128, Cm * u:Cm * (u + 1)] = w1f[:, :, u, 1].T
            a[0:C1, 3 * Cm + Cm * u:3 * Cm + Cm * (u + 1)] = w1f[:, :, u, 2].T
            a[0:C1, 6 * Cm + Cm * u:6 * Cm + Cm * (u + 1)] = w1f[:, :, u, 1].T
            for j in range(3):
                s2p[Cm * j:Cm * (j + 1), C2 * u:C2 * (u + 1)] = w2f[:, :, u, j].T
        return a.astype(bf), s2p.astype(bf)

    wp = {0: packs(w1, w2), 1: packs(w1[:, :, ::-1, :], w2[:, :, ::-1, :])}

    in_maps = []
    for core in range(NCORES):
        b, fl = core // 2, core % 2
        xb = x[b] if fl == 0 else x[b][:, ::-1, :]
        xpad = np.zeros((C1, HP, WP), f32)
        xpad[:, 1:H + 1, 1:W + 1] = xb
        m = dict(common)
        m["xs"] = xpad.reshape(C1, NP).astype(np.float16)
        w1a, w2s = wp[fl]
        m["w1pack"] = w1a
        w2full = w2pack.copy()
        w2full[:, 0:3 * C2] = w2s
        m["w2pack"] = w2full
        in_maps.append(m)
    return in_maps


def _assemble(results):
    out = np.empty((B, C2, H, W), np.float32)
    for core in range(NCORES):
        b, fl = core // 2, core % 2
        o = results[core]["out"].reshape(C2, H // 2, W)
        if fl == 0:
            out[b, :, 0:H // 2, :] = o
        else:
            out[b, :, H // 2:H, :] = o[:, ::-1, :]
    return out


def _run(inputs, trace=False):
    nc = _build_program()
    in_maps = _host_prep(inputs)
    res = run_bass_kernel_spmd(nc, in_maps, core_ids=list(range(NCORES)), trace=trace)
    return _assemble(res.results), res


def kernel(**inputs):
    out, _ = _run(inputs)
    return out


# revision 52
# speedup vs baseline: 1.1183x; 1.1183x over previous
"""Trainium2 Bass kernel for nn_Bottleneck_SAA (CSP bottleneck + dual PAM attention).

Sharding: 8 cores = 4 batches x 2 row-halves. One SPMD program; odd cores
receive a vertically flipped image + vertically flipped conv kernels, so
every core computes output rows 0..31 of its (possibly flipped) input
(conv(flip(x), flip_h(w)) == flip(conv(x, w)); attention is invariant to
permuting the softmax axis). The host flips those outputs back.

Per-core on-chip pipeline (fp16 matmul operands, fp32 PSUM accumulate):
  conv1(3x3, BN+SiLU folded into weights/bias) -> conv2 -> q/k/vT
  projections -> flash-style attention in [m, n] orientation:
    energyT = k_chunk^T q  (PSUM) -> exp (ACT, fp8 out) ->
    outT[65, n] += [vT | 1]^T expT   (row 64 = softmax denominator)
  normalization via DVE reciprocal + K=1 ones-matmul partition broadcast;
  residual fused: out = x + 2*y + 2*gamma*(attn_out + v_b).  2*gamma is
  folded into vwT host-side; 2*gamma*v_b rides a bias row appended to the
  v projection (the softmax denominator cancels it exactly), so the
  residual input x is re-read straight out of the xs conv stack -- no
  separate xh input tensor.

v3 scheduling changes (141.9us baseline):
  - HAM warm-up: the PE clock-gates to 1.2 GHz unless the ARRAY (not the
    instruction stream -- v2 measured 100%-issue-busy K=8 matmuls staying
    cold for 106us) is kept active.  The K=8 energy matmuls light up only
    8/128 rows, so they never register as busy.  Fix: zero-pad the
    energy contraction to K=128 (k_ext/q_ext rows 8-127 are zeros --
    exact, and the 512-cycle stream cost is K-independent).  The conv
    phase (K=128/96) already holds K=8/8 warm (v2: 18.5->47us).
  - explicit keep-warm dummy matmuls are a NET LOSS: interleaving
    different array configs (col_grp/row_grp/perf-mode switches) defeats
    the drain/fill overlap between consecutive matmuls (+250ns/MM,
    measured v2) and they still don't register occupancy.  Don't.
  - startup: the first conv matmul is HBM-bandwidth-bound, so the xs
    stack ships only the x half (557KB); the [x<<1] copy is made
    on-chip by 4 chunked SBUF->SBUF shift DMAs.  Weights packed into two
    DMA blobs; xs split into 4 column chunks across the
    sync/scalar/gpsimd queues; conv1's first group shrunk to 2 tiles so
    it only waits for xs cols 0:1056.
  - final out DMAs for the last half split across queues (the ~2us
    HBM-write receipt latency parallelizes).

Conv tricks: every pass streams ONE contiguous span of the zero-padded
[66x66] image (row-tap start offsets keep passes column-aligned in
PSUM; pad-column garbage is skipped at SiLU evacuation). Column taps
(u,0)/(u,1) ride the CONTRACTION axis via the host-built [x, x<<1]
stack; conv2 stacks [y1, y1<<1, y1<<2] on 96 partitions so all 3 column
taps fuse into K=96 (3 passes instead of 9). conv1 reads ONLY the
[x, x<<1] stack: taps (u,2) come from streaming the x block at a +2
column offset (3x K=128 + 3x K=64 passes).
"""

import sys

sys.path.insert(0, "/opt/trn_rl_repo")

from contextlib import ExitStack

import numpy as np
import ml_dtypes

import concourse.bass as bass
import concourse.tile as tile
from concourse import bacc, mybir
from concourse.bass_utils import run_bass_kernel_spmd

B, C1, C2, Cm, C8 = 4, 64, 64, 32, 8
H = W = 64
N = H * W            # 4096 pixels
NH = N // 2          # 2048 pixels per core (32 rows)
HP = H + 2           # padded height
WP = W + 2
NP = HP * WP         # 4356
NCORES = 8
EPS = 1e-5
FP32 = mybir.dt.float32
AF = mybir.ActivationFunctionType
ALU = mybir.AluOpType

MCHUNKS = N // 128   # 32 attention m-chunks
NSPAN = 1024         # n columns processed per accumulator half
BF16 = mybir.dt.float16  # 16-bit matmul operand dtype (fp16: 8x less rounding than bf16)
FP8 = mybir.dt.float8e4   # e4m3 for the attention-weights matmul (DoubleRow)
VP = 80                   # padded per-chunk lhsT columns (65 -> 80, step%16==0)
RPT = 7              # conv: image rows per matmul (contiguous-stream tiling)

# xs column chunk boundaries (one DMA per chunk, one queue each)
XCH = [0, 16 * WP, 32 * WP, 49 * WP, NP]

_build_cache = {}


def _build_program():
    if "nc" in _build_cache:
        return _build_cache["nc"]
    nc = bacc.Bacc("TRN2", target_bir_lowering=False, debug=False, num_devices=NCORES)

    xp_d = nc.dram_tensor("xs", [C1, NP], BF16, kind="ExternalInput")
    w1_d = nc.dram_tensor("w1pack", [128, 3 * 3 * Cm], BF16, kind="ExternalInput")
    b1_d = nc.dram_tensor("b1", [Cm, 1], FP32, kind="ExternalInput")
    w2_d = nc.dram_tensor("w2pack", [96, 3 * C2 + 2 * C8 + C2], BF16, kind="ExternalInput")
    b2_d = nc.dram_tensor("b2", [C2, 1], FP32, kind="ExternalInput")
    out_d = nc.dram_tensor("out", [C2, NH], FP32, kind="ExternalOutput")

    with tile.TileContext(nc) as tc:
        with ExitStack() as ctx:
            per = ctx.enter_context(tc.tile_pool(name="persist", bufs=1))

            xs_sb = per.tile([128, NP], BF16)
            w1p_sb = per.tile([128, 3 * 3 * Cm], BF16)
            b1_sb = per.tile([Cm, 1], FP32)
            w2p_sb = per.tile([96, 3 * C2 + 2 * C8 + C2], BF16)
            b2_sb = per.tile([C2, 1], FP32)
            ones_sb = per.tile([1, C2], BF16)

            w1_sb = w1p_sb[:, 0:3 * Cm]
            w1b_sb = w1p_sb[0:C1, 3 * Cm:6 * Cm]
            w1c_sb = w1p_sb[0:C1, 6 * Cm:9 * Cm]   # tap (u,1) at partition base 0
            w2_sb = w2p_sb[0:96, 0:3 * C2]
            qw_sb = w2p_sb[0:C2 + 1, 3 * C2:3 * C2 + C8]
            kw_sb = w2p_sb[0:C2 + 1, 3 * C2 + C8:3 * C2 + 2 * C8]
            vw_sb = w2p_sb[0:C2 + 1, 3 * C2 + 2 * C8:3 * C2 + 2 * C8 + C2]

            ys_sb = per.tile([96, NP], BF16)       # conv1 out + 2 column-shifted copies
            y_sb = per.tile([C2 + 1, N], BF16)     # conv2 output, row 64 = ones (bias)
            k_sb = per.tile([128, N], BF16)        # rows 0-7 = k, rows 8-127 = 0
            q_sb = per.tile([128, NH], BF16)       # rows 0-7 = q, rows 8-127 = 0
            vext_sb = per.tile([128, (MCHUNKS // 2) * 2 * VP], FP8)  # [128, 16, 2, 80]
            r_sb = per.tile([C2, NH], FP32)        # x_half + 2*y_half
            fin_sb = per.tile([C2, NH], FP32)

            # startup DMA plan: the conv1 critical set (w1pack + xs chunk 0)
            # leads two different queues; the [x<<1] stack half is built
            # on-chip by chunked SBUF->SBUF shift DMAs (x<<1 from HBM would
            # double the startup HBM traffic).  A shift trigger WAITS on its
            # source chunk's semaphore and blocks everything behind it on
            # its engine queue, so shifts are issued LAST per queue.  conv1
            # group 0 uses the 9-pass x-only form, so no shift gates it.
            nc.sync.dma_start(xs_sb[0:C1, XCH[0]:XCH[1]], xp_d.ap()[:, XCH[0]:XCH[1]])
            nc.scalar.dma_start(w1p_sb[:], w1_d.ap())
            nc.gpsimd.dma_start(xs_sb[0:C1, XCH[2]:XCH[3]], xp_d.ap()[:, XCH[2]:XCH[3]])
            nc.scalar.dma_start(xs_sb[0:C1, XCH[1]:XCH[2]], xp_d.ap()[:, XCH[1]:XCH[2]])
            nc.sync.dma_start(xs_sb[0:C1, XCH[3]:XCH[4]], xp_d.ap()[:, XCH[3]:XCH[4]])
            nc.sync.dma_start(w2p_sb[:], w2_d.ap())
            nc.scalar.dma_start(b1_sb[:], b1_d.ap())
            nc.scalar.dma_start(b2_sb[:], b2_d.ap())
            # shift DMAs (dest chunk c reads source cols +1, all within the
            # same source chunk; boundaries offset by one column)
            nc.sync.dma_start(xs_sb[C1:128, XCH[0]:XCH[1] - 1],
                              xs_sb[0:C1, XCH[0] + 1:XCH[1]])
            nc.scalar.dma_start(xs_sb[C1:128, XCH[1] - 1:XCH[2] - 1],
                                xs_sb[0:C1, XCH[1]:XCH[2]])
            nc.gpsimd.dma_start(xs_sb[C1:128, XCH[2] - 1:XCH[3] - 1],
                                xs_sb[0:C1, XCH[2]:XCH[3]])
            nc.sync.dma_start(xs_sb[C1:128, XCH[3] - 1:XCH[4] - 1],
                              xs_sb[0:C1, XCH[3]:XCH[4]])

            # zero the padded contraction rows of q/k ONCE (both sides:
            # 0 * NaN-garbage would still poison the energy sums).  Whole
            # tile -- engine APs need 32-aligned partition bases; the
            # projections overwrite rows 0-7 later.
            nc.vector.memset(q_sb[:], 0.0)
            nc.vector.memset(k_sb[:], 0.0)

            nc.gpsimd.memset(ones_sb[:], 1.0)
            nc.gpsimd.memset(y_sb[C2:C2 + 1, :], 1.0)

            ys_v = ys_sb[:].rearrange("p (a b) -> p a b", b=WP)
            # zero only what conv2 actually reads as padding: rows 0/65
            # everywhere, pad cols 0/65 on the unshifted block (the shifted
            # blocks inherit correct pads from the chunked DMAs).
            nc.gpsimd.memset(ys_sb[:, 0:WP], 0.0)
            nc.gpsimd.memset(ys_sb[:, (HP - 1) * WP:NP], 0.0)
            nc.gpsimd.memset(ys_v[0:Cm, 1:HP - 1, 0:1], 0.0)
            nc.gpsimd.memset(ys_v[0:Cm, 1:HP - 1, WP - 1:WP], 0.0)

            vext_v = vext_sb[:].rearrange("p (c s k) -> p c s k", s=2, k=VP)
            nc.gpsimd.memset(vext_v[:, :, :, C2:], 0.0)
            nc.gpsimd.memset(vext_v[:, :, :, C2:C2 + 1], 1.0)

            xs_v = xs_sb[:].rearrange("p (a b) -> p a b", b=WP)
            y_v = y_sb[0:C2, :]
            y_rows = y_sb[0:C2, :].rearrange("p (a b) -> p a b", b=W)
            r_rows = r_sb[:].rearrange("p (a b) -> p a b", b=W)
            y65 = y_sb[:]

            # conv tiling: groups of RPT image rows; each tap streams one
            # CONTIGUOUS span of the padded image (garbage at the 2 pad
            # columns per row accumulates in psum and is skipped on
            # evacuation).
            conv_tiles = [(RPT * t, RPT) for t in range(H // RPT)]
            if H % RPT:
                conv_tiles.append((H - H % RPT, H % RPT))

            # conv1: 6 streamed passes/tile (the contraction-lower-bound):
            # 3x K=128 on xs=[x, x<<1] (taps (u,0)+(u,1)), 3x K=64 on the x
            # block at a +2 column offset (taps (u,2)).  Group 0 instead
            # runs 9x K=64 on the x block alone (taps (u,c) at +c column
            # offsets) so it isn't gated on the on-chip x<<1 shift DMA.
            def conv1_group(psA, tiles, nine=False):
                ps = psA.tile([128, WP * RPT], FP32, tag="mm")
                def mm(g, pass_i):
                    r0, nr = tiles[g]
                    length = WP * (nr - 1) + W
                    pslc = ps[Cm * g:Cm * (g + 1), 0:length]
                    tp = (0, Cm * g)
                    if nine:
                        u, c = pass_i // 3, pass_i % 3
                        lhs = (w1_sb[0:C1, Cm * u:Cm * (u + 1)] if c == 0 else
                               w1c_sb[:, Cm * u:Cm * (u + 1)] if c == 1 else
                               w1b_sb[:, Cm * u:Cm * (u + 1)])
                        s = (r0 + u) * WP + c
                        nc.tensor.matmul(
                            pslc, lhs, xs_sb[0:C1, s:s + length],
                            start=(pass_i == 0), stop=(pass_i == 8),
                            tile_position=tp,
                        )
                    elif pass_i < 3:
                        s = (r0 + pass_i) * WP
                        nc.tensor.matmul(
                            pslc, w1_sb[:, Cm * pass_i:Cm * (pass_i + 1)],
                            xs_sb[:, s:s + length],
                            start=(pass_i == 0), stop=False, tile_position=tp,
                        )
                    else:
                        u = pass_i - 3
                        s = (r0 + u) * WP + 2
                        nc.tensor.matmul(
                            pslc, w1b_sb[:, Cm * u:Cm * (u + 1)],
                            xs_sb[0:C1, s:s + length],
                            start=False, stop=(pass_i == 5), tile_position=tp,
                        )
                for pass_i in range(9 if nine else 6):
                    for g in range(len(tiles)):
                        mm(g, pass_i)
                ps_v = ps[:].rearrange("p (r w) -> p r w", w=WP)
                for g, (r0, nr) in enumerate(tiles):
                    nc.scalar.activation(
                        ys_v[0:Cm, 1 + r0:1 + r0 + nr, 1:1 + W],
                        ps_v[Cm * g:Cm * (g + 1), 0:nr, 0:W],
                        AF.Silu, bias=b1_sb[:, 0:1],
                    )

            # conv2: all 3 column taps on the partition axis (K=96, shifted
            # copies of y1 at rows 32-63 / 64-95): 3 passes instead of 9.
            def conv2_group(psA, tiles):
                ps = psA.tile([128, WP * RPT], FP32, tag="mm")
                for u in range(3):
                    for g, (r0, nr) in enumerate(tiles):
                        length = WP * (nr - 1) + W
                        s = (r0 + u) * WP
                        nc.tensor.matmul(
                            ps[C2 * g:C2 * (g + 1), 0:length],
                            w2_sb[:, C2 * u:C2 * (u + 1)],
                            ys_sb[:, s:s + length], start=(u == 0), stop=(u == 2),
                            tile_position=(0, C2 * g),
                        )
                ps_v = ps[:].rearrange("p (r w) -> p r w", w=WP)
                for g, (r0, nr) in enumerate(tiles):
                    nc.scalar.activation(
                        y_rows[:, r0:r0 + nr, :],
                        ps_v[C2 * g:C2 * (g + 1), 0:nr, 0:W],
                        AF.Silu, bias=b2_sb[:, 0:1],
                    )

            with tc.tile_pool(name="psA", bufs=6, space="PSUM") as psA:
                # conv1 with the y1 column-shift DMAs chunked in right behind
                # the tiles that produce their source rows.  First group is 2
                # tiles so it starts as soon as xs chunk 0 (cols 0:1056) lands.
                c1groups = [conv_tiles[0:2], conv_tiles[2:6], conv_tiles[6:10]]
                for gi, grp in enumerate(c1groups):
                    conv1_group(psA, grp, nine=(gi == 0))
                    for c0 in range(0, len(grp), 2):
                        sub = grp[c0:c0 + 2]
                        a = (1 + sub[0][0]) * WP
                        b = (1 + sub[-1][0] + sub[-1][1]) * WP
                        nc.sync.dma_start(ys_sb[Cm:2 * Cm, a - 1:b - 1],
                                          ys_sb[0:Cm, a:b])
                        nc.gpsimd.dma_start(ys_sb[2 * Cm:3 * Cm, a - 2:b - 2],
                                            ys_sb[0:Cm, a:b])

                # conv2 with projections (and the residual) interleaved; a
                # projection issues one tile AFTER its y span completes so the
                # in-order PE never parks on a SiLU that hasn't drained yet.
                k_done = q_done = vg_done = r_done = 0
                groups = [conv_tiles[i:i + 2] for i in range(0, len(conv_tiles), 2)]
                for grp in groups + [None]:
                    if grp is not None:
                        conv2_group(psA, grp)
                    rows = grp[0][0] if grp is not None else H
                    while q_done < min(4, rows // 8):
                        t0 = q_done
                        ps = psA.tile([C8, 512], FP32, tag="mm")
                        nc.tensor.matmul(ps[:], qw_sb[:],
                                         y65[:, 512 * t0:512 * (t0 + 1)],
                                         start=True, stop=True)
                        nc.vector.tensor_copy(q_sb[0:C8, 512 * t0:512 * (t0 + 1)],
                                              ps[:])
                        q_done += 1
                    while k_done < min(8, rows // 8):
                        t0 = k_done
                        ps = psA.tile([C8, 512], FP32, tag="mm")
                        nc.tensor.matmul(ps[:], kw_sb[:],
                                         y65[:, 512 * t0:512 * (t0 + 1)],
                                         start=True, stop=True)
                        nc.vector.tensor_copy(k_sb[0:C8, 512 * t0:512 * (t0 + 1)],
                                              ps[:])
                        k_done += 1
                    while vg_done < rows // 16:
                        g = vg_done
                        ps = psA.tile([128, 512], FP32, tag="mm")
                        for i in range(8):
                            j = 8 * g + i
                            nc.tensor.matmul(
                                ps[:, C2 * i:C2 * (i + 1)],
                                y65[:, 128 * j:128 * (j + 1)],
                                vw_sb[:],
                                start=True, stop=True,
                            )
                        nc.vector.tensor_copy(vext_v[:, 4 * g:4 * (g + 1), :, 0:C2],
                                              ps[:])
                        vg_done += 1
                    while r_done < min(2, rows // 16):
                        c = r_done
                        rs = slice(16 * c, 16 * (c + 1))
                        nc.vector.scalar_tensor_tensor(
                            r_rows[:, rs, :], y_rows[:, rs, :], 2.0,
                            xs_v[0:C1, 1 + 16 * c:1 + 16 * (c + 1), 1:1 + W],
                            ALU.mult, ALU.add,
                        )
                        r_done += 1

            # ---- attention, two sequential 1024-col halves ----
            with (
                tc.tile_pool(name="psE", bufs=3, space="PSUM") as psE,
                tc.tile_pool(name="psO", bufs=1, space="PSUM") as psO,
                tc.tile_pool(name="expp", bufs=4) as expp,
            ):
                for nh in range(2):
                    cs = NSPAN * nh
                    po = psO.tile([128, NSPAN], FP32, tag="po")
                    pending = []
                    dr_rem = []
                    for p in range(MCHUNKS // 2):
                        exv = expp.tile([128, 2 * NSPAN], FP8, tag="ex")
                        exv = exv[:].rearrange("q (s n) -> q s n", s=2)
                        for s in range(2):
                            i = 2 * p + s
                            pe = psE.tile([128, NSPAN], FP32, tag="pe")
                            for j in range(2):
                                # sandwich each lagged DR matmul between
                                # energy matmuls so its LDWEIGHTS hides
                                # under the neighbour's streaming
                                if dr_rem:
                                    _mm1(nc, po, vext_v, *dr_rem.pop(0))
                                nc.tensor.matmul(
                                    pe[:, 512 * j:512 * (j + 1)],
                                    k_sb[:, 128 * i:128 * (i + 1)],
                                    q_sb[:, cs + 512 * j:cs + 512 * (j + 1)],
                                    start=True, stop=True,
                                )
                            nc.scalar.activation(exv[:, s, :], pe[:], AF.Exp)
                        pending.append((exv, p))
                        if len(pending) > 3:
                            e0, p0 = pending.pop(0)
                            dr_rem += [(e0, p0, 0), (e0, p0, 1)]
                    for t in dr_rem:
                        _mm1(nc, po, vext_v, *t)
                    for e0, p0 in pending:
                        _mm1(nc, po, vext_v, e0, p0, 0)
                        _mm1(nc, po, vext_v, e0, p0, 1)

                    # ---- epilogue for this half, pipelined in 512-col blocks;
                    # all elementwise work on the (idle) DVE so ACT stays on
                    # the exp table for the next half's attention.
                    # 1/sumexp: bounce the [1,1024] denominator row through a
                    # [32,32] tile via DMA so the (per-column-costed) DVE
                    # reciprocal runs over 32 columns (0.36us) instead of 1024.
                    o_sb = per.tile([C2 + 1, NSPAN], FP32, tag="o_sb")
                    rec16 = per.tile([1, NSPAN], BF16, tag="rec16")
                    t1 = per.tile([C2, NSPAN], FP32, tag="t1")
                    zt0 = per.tile([16, 32], FP32, tag="zt0")
                    zr0 = per.tile([16, 32], FP32, tag="zr0")
                    zc0 = per.tile([16, 32], BF16, tag="zc0")
                    zt1 = per.tile([16, 32], FP32, tag="zt1")
                    zr1 = per.tile([16, 32], FP32, tag="zr1")
                    zc1 = per.tile([16, 32], BF16, tag="zc1")
                    zs = [(zt0, zr0, zc0), (zt1, zr1, zc1)]
                    # per-j denominator chains: the two SBUF bounce-DMA
                    # latencies pipeline instead of serializing.  Split
                    # [65,512] evacuations: rows 0-63 = numerator, row 64 =
                    # denominator (a separate [1,1024] copy costs a full
                    # 1024 per-column DVE pass on its own).
                    for j in range(2):
                        sl = slice(512 * j, 512 * (j + 1))
                        nc.vector.tensor_copy(o_sb[:, sl], po[0:C2 + 1, sl])
                        nc.sync.dma_start(zs[j][0][:], o_sb[C2:C2 + 1, sl])
                    for j in range(2):
                        sl = slice(512 * j, 512 * (j + 1))
                        nc.vector.reciprocal(zs[j][1][:], zs[j][0][:])
                        nc.vector.tensor_copy(zs[j][2][:], zs[j][1][:])
                        nc.sync.dma_start(rec16[:, sl], zs[j][2][:])
                    for j in range(2):
                        sl = slice(512 * j, 512 * (j + 1))
                        gl = slice(cs + 512 * j, cs + 512 * (j + 1))
                        nc.tensor.matmul(po[0:C2, sl], ones_sb[:], rec16[:, sl],
                                         start=True, stop=True)
                        nc.vector.tensor_mul(t1[:, sl], o_sb[0:C2, sl], po[0:C2, sl])
                        nc.vector.tensor_add(fin_sb[:, gl], t1[:, sl], r_sb[:, gl])
                        if nh == 0:
                            # sync only: the scalar queue is the ACT engine --
                            # a DMA trigger there would stall half 2's exps.
                            nc.sync.dma_start(out_d.ap()[:, gl], fin_sb[:, gl])
                        else:
                            ga, gb = cs + 512 * j, cs + 512 * j + 256
                            e0, e1 = (nc.sync, nc.gpsimd) if j == 0 else (
                                nc.scalar, nc.gpsimd)
                            e0.dma_start(out_d.ap()[:, ga:gb], fin_sb[:, ga:gb])
                            e1.dma_start(out_d.ap()[:, gb:gb + 256],
                                         fin_sb[:, gb:gb + 256])

    nc.compile()
    _build_cache["nc"] = nc
    return nc


def _mm1(nc, po, vext_v, exv, p, j):
    # DoubleRow fp8: contract 256 m-rows (chunk pair 2p, 2p+1) per pass.
    # po[m, n] += sum_s vext_{2p+s}[:, m]^T expT_{2p+s}[:, n]; row C2 = sum(exp)
    nc.tensor.matmul(
        po[0:VP, 512 * j:512 * (j + 1)],
        vext_v[:, p, :, :],
        exv[:, :, 512 * j:512 * (j + 1)],
        start=(p == 0), stop=(p == MCHUNKS // 2 - 1),
        perf_mode=mybir.MatmulPerfMode.DoubleRow,
    )


def _host_prep(inputs):
    f32 = np.float32
    x = np.asarray(inputs["x"], f32)
    s1 = np.asarray(inputs["bn1_g"], f32) / np.sqrt(np.asarray(inputs["bn1_v"], f32) + EPS)
    bb1 = np.asarray(inputs["bn1_b"], f32) - np.asarray(inputs["bn1_m"], f32) * s1
    w1 = np.asarray(inputs["cv1_w"], f32) * s1[:, None, None, None]
    s2 = np.asarray(inputs["bn2_g"], f32) / np.sqrt(np.asarray(inputs["bn2_v"], f32) + EPS)
    bb2 = np.asarray(inputs["bn2_b"], f32) - np.asarray(inputs["bn2_m"], f32) * s2
    w2 = np.asarray(inputs["cv2_w"], f32) * s2[:, None, None, None]
    gamma = f32(np.asarray(inputs["pam_gamma"], f32))

    bf = np.float16

    def aug(w, b):
        # [Co, C2] weight + [Co] bias -> [C2+1, Co] lhsT with bias row
        co = np.asarray(w, f32).shape[0]
        a = np.zeros((C2 + 1, co), f32)
        a[0:C2, :] = np.asarray(w, f32).T
        a[C2, :] = np.asarray(b, f32)
        return a.astype(bf)

    # w2pack: [96, 3*C2 | qwa(8) | kwa(8) | vwB(64)]
    w2pack = np.zeros((96, 3 * C2 + 2 * C8 + C2), bf)
    w2pack[0:C2 + 1, 3 * C2:3 * C2 + C8] = aug(inputs["q_w"], inputs["q_b"])
    w2pack[0:C2 + 1, 3 * C2 + C8:3 * C2 + 2 * C8] = aug(inputs["k_w"], inputs["k_b"])
    w2pack[0:C2 + 1, 3 * C2 + 2 * C8:] = aug(
        2.0 * gamma * np.asarray(inputs["v_w"], f32),
        2.0 * gamma * np.asarray(inputs["v_b"], f32))

    common = {
        "b1": np.ascontiguousarray(bb1[:, None]),
        "b2": np.ascontiguousarray(bb2[:, None]),
    }

    def packs(w1f, w2f):
        a = np.zeros((128, 9 * Cm), np.float32)
        s2p = np.zeros((96, 3 * C2), np.float32)
        for u in range(3):
            a[0:C1, Cm * u:Cm * (u + 1)] = w1f[:, :, u, 0].T
            a[C1:128, Cm * u:Cm * (u + 1)] = w1f[:, :, u, 1].T
            a[0:C1, 3 * Cm + Cm * u:3 * Cm + Cm * (u + 1)] = w1f[:, :, u, 2].T
            a[0:C1, 6 * Cm + Cm * u:6 * Cm + Cm * (u + 1)] = w1f[:, :, u, 1].T
            for j in range(3):
                s2p[Cm * j:Cm * (j + 1), C2 * u:C2 * (u + 1)] = w2f[:, :, u, j].T
        return a.astype(bf), s2p.astype(bf)

    wp = {0: packs(w1, w2), 1: packs(w1[:, :, ::-1, :], w2[:, :, ::-1, :])}

    in_maps = []
    for core in range(NCORES):
        b, fl = core // 2, core % 2
        xb = x[b] if fl == 0 else x[b][:, ::-1, :]
        xpad = np.zeros((C1, HP, WP), f32)
        xpad[:, 1:H + 1, 1:W + 1] = xb
        m = dict(common)
        m["xs"] = xpad.reshape(C1, NP).astype(np.float16)
        w1a, w2s = wp[fl]
        m["w1pack"] = w1a
        w2full = w2pack.copy()
        w2full[:, 0:3 * C2] = w2s
        m["w2pack"] = w2full
        in_maps.append(m)
    return in_maps


def _assemble(results):
    out = np.empty((B, C2, H, W), np.float32)
    for core in range(NCORES):
        b, fl = core // 2, core % 2
        o = results[core]["out"].reshape(C2, H // 2, W)
        if fl == 0:
            out[b, :, 0:H // 2, :] = o
        else:
            out[b, :, H // 2:H, :] = o[:, ::-1, :]
    return out


def _run(inputs, trace=False):
    nc = _build_program()
    in_maps = _host_prep(inputs)
    res = run_bass_kernel_spmd(nc, in_maps, core_ids=list(range(NCORES)), trace=trace)
    return _assemble(res.results), res


def kernel(**inputs):
    out, _ = _run(inputs)
    return out


# revision 56
# speedup vs baseline: 1.1322x; 1.0124x over previous
"""Trainium2 Bass kernel for nn_Bottleneck_SAA (CSP bottleneck + dual PAM attention).

Sharding: 8 cores = 4 batches x 2 row-halves. One SPMD program; odd cores
receive a vertically flipped image + vertically flipped conv kernels, so
every core computes output rows 0..31 of its (possibly flipped) input
(conv(flip(x), flip_h(w)) == flip(conv(x, w)); attention is invariant to
permuting the softmax axis). The host flips those outputs back.

Per-core on-chip pipeline (fp16 matmul operands, fp32 PSUM accumulate):
  conv1(3x3, BN+SiLU folded into weights/bias) -> conv2 -> q/k/vT
  projections -> flash-style attention in [m, n] orientation:
    energyT = k_chunk^T q  (PSUM) -> exp (ACT, fp8 out) ->
    outT[65, n] += [vT | 1]^T expT   (row 64 = softmax denominator)
  normalization via DVE reciprocal + K=1 ones-matmul partition broadcast;
  residual fused: out = x + 2*y + 2*gamma*(attn_out + v_b).  2*gamma is
  folded into vwT host-side; 2*gamma*v_b rides a bias row appended to the
  v projection (the softmax denominator cancels it exactly), so the
  residual input x is re-read straight out of the xs conv stack -- no
  separate xh input tensor.

Scheduling (141.9us baseline -> 120.3us measured):
  - HAM warm-up: the PE clock-gates to 1.2 GHz unless the ARRAY (not the
    instruction stream -- v2 measured 100%-issue-busy K=8 matmuls staying
    cold for 106us) is kept active.  The K=8 energy matmuls light up only
    8/128 rows, so they never register as busy.  Fix: zero-pad the
    energy contraction to K=128 (k_ext/q_ext rows 8-127 are zeros --
    exact, and the 512-cycle stream cost is K-independent).  The conv
    phase (K=128/96) already holds K=8/8 warm (v2: 18.5->47us).
  - explicit keep-warm dummy matmuls are a NET LOSS: interleaving
    different array configs (col_grp/row_grp/perf-mode switches) defeats
    the drain/fill overlap between consecutive matmuls (+250ns/MM,
    measured v2) and they still don't register occupancy.  Don't.
  - startup: the first conv matmul is HBM-bandwidth-bound, so the xs
    stack ships only the x half (557KB); the [x<<1] copy is made
    on-chip by 4 chunked SBUF->SBUF shift DMAs.  Weights packed into two
    DMA blobs; xs split into 4 column chunks across the
    sync/scalar/gpsimd queues; conv1's first group shrunk to 2 tiles so
    it only waits for xs cols 0:1056.
  - final out DMAs for the last half split across queues (the ~2us
    HBM-write receipt latency parallelizes); epilogue denominator
    bounce chains split per 512-col block so their two SBUF-DMA
    latencies pipeline.
  - tried and REJECTED (all measured slower): interleaving attention
    pairs into the conv2 pipeline (PE-bound at cold clock + table
    thrash: SILU and EXP live in different ACT table sets, every
    interleave boundary reloads 1.28us; a tanh-based silu in exp's set
    fixes tables but makes the DVE the bottleneck), keep-warm dummy
    matmuls, single_packet on partition-crossing DMAs (device crash).

Conv tricks: every pass streams ONE contiguous span of the zero-padded
[66x66] image (row-tap start offsets keep passes column-aligned in
PSUM; pad-column garbage is skipped at SiLU evacuation). Column taps
(u,0)/(u,1) ride the CONTRACTION axis via the host-built [x, x<<1]
stack; conv2 stacks [y1, y1<<1, y1<<2] on 96 partitions so all 3 column
taps fuse into K=96 (3 passes instead of 9). conv1 reads ONLY the
[x, x<<1] stack: taps (u,2) come from streaming the x block at a +2
column offset (3x K=128 + 3x K=64 passes).
"""

import sys

sys.path.insert(0, "/opt/trn_rl_repo")

from contextlib import ExitStack

import numpy as np
import ml_dtypes

import concourse.bass as bass
import concourse.tile as tile
from concourse import bacc, mybir
from concourse.bass_utils import run_bass_kernel_spmd

B, C1, C2, Cm, C8 = 4, 64, 64, 32, 8
H = W = 64
N = H * W            # 4096 pixels
NH = N // 2          # 2048 pixels per core (32 rows)
HP = H + 2           # padded height
WP = W + 2
NP = HP * WP         # 4356
NCORES = 8
EPS = 1e-5
FP32 = mybir.dt.float32
AF = mybir.ActivationFunctionType
ALU = mybir.AluOpType

MCHUNKS = N // 128   # 32 attention m-chunks
NSPAN = 1024         # n columns processed per accumulator half
BF16 = mybir.dt.float16  # 16-bit matmul operand dtype (fp16: 8x less rounding than bf16)
FP8 = mybir.dt.float8e4   # e4m3 for the attention-weights matmul (DoubleRow)
VP = 80                   # padded per-chunk lhsT columns (65 -> 80, step%16==0)
RPT = 7              # conv: image rows per matmul (contiguous-stream tiling)

# xs column chunk boundaries (one DMA per chunk, one queue each)
XCH = [0, 16 * WP, 32 * WP, 49 * WP, NP]

_build_cache = {}


def _build_program():
    if "nc" in _build_cache:
        return _build_cache["nc"]
    nc = bacc.Bacc("TRN2", target_bir_lowering=False, debug=False, num_devices=NCORES)

    xp_d = nc.dram_tensor("xs", [C1, NP], BF16, kind="ExternalInput")
    w1_d = nc.dram_tensor("w1pack", [128, 3 * 3 * Cm], BF16, kind="ExternalInput")
    b1_d = nc.dram_tensor("b1", [Cm, 1], FP32, kind="ExternalInput")
    w2_d = nc.dram_tensor("w2pack", [96, 3 * C2 + 2 * C8 + C2], BF16, kind="ExternalInput")
    b2_d = nc.dram_tensor("b2", [C2, 1], FP32, kind="ExternalInput")
    out_d = nc.dram_tensor("out", [C2, NH], FP32, kind="ExternalOutput")

    with tile.TileContext(nc) as tc:
        with ExitStack() as ctx:
            per = ctx.enter_context(tc.tile_pool(name="persist", bufs=1))

            xs_sb = per.tile([128, NP], BF16)
            w1p_sb = per.tile([128, 3 * 3 * Cm], BF16)
            b1_sb = per.tile([Cm, 1], FP32)
            w2p_sb = per.tile([96, 3 * C2 + 2 * C8 + C2], BF16)
            b2_sb = per.tile([C2, 1], FP32)
            ones_sb = per.tile([1, C2], BF16)

            w1_sb = w1p_sb[:, 0:3 * Cm]
            w1b_sb = w1p_sb[0:C1, 3 * Cm:6 * Cm]
            w1c_sb = w1p_sb[0:C1, 6 * Cm:9 * Cm]   # tap (u,1) at partition base 0
            w2_sb = w2p_sb[0:96, 0:3 * C2]
            qw_sb = w2p_sb[0:C2 + 1, 3 * C2:3 * C2 + C8]
            kw_sb = w2p_sb[0:C2 + 1, 3 * C2 + C8:3 * C2 + 2 * C8]
            vw_sb = w2p_sb[0:C2 + 1, 3 * C2 + 2 * C8:3 * C2 + 2 * C8 + C2]

            ys_sb = per.tile([96, NP], BF16)       # conv1 out + 2 column-shifted copies
            y_sb = per.tile([C2 + 1, N], BF16)     # conv2 output, row 64 = ones (bias)
            k_sb = per.tile([128, N], BF16)        # rows 0-7 = k, rows 8-127 = 0
            q_sb = per.tile([128, NH], BF16)       # rows 0-7 = q, rows 8-127 = 0
            vext_sb = per.tile([128, (MCHUNKS // 2) * 2 * VP], FP8)  # [128, 16, 2, 80]
            r_sb = per.tile([C2, NH], FP32)        # x_half + 2*y_half
            fin_sb = per.tile([C2, NH], FP32)

            # startup DMA plan: the conv1 critical set (w1pack + xs chunk 0)
            # leads two different queues; the [x<<1] stack half is built
            # on-chip by chunked SBUF->SBUF shift DMAs (x<<1 from HBM would
            # double the startup HBM traffic).  A shift trigger WAITS on its
            # source chunk's semaphore and blocks everything behind it on
            # its engine queue, so shifts are issued LAST per queue.  conv1
            # group 0 uses the 9-pass x-only form, so no shift gates it.
            nc.sync.dma_start(xs_sb[0:C1, XCH[0]:XCH[1]], xp_d.ap()[:, XCH[0]:XCH[1]])
            nc.scalar.dma_start(w1p_sb[:], w1_d.ap())
            nc.gpsimd.dma_start(xs_sb[0:C1, XCH[2]:XCH[3]], xp_d.ap()[:, XCH[2]:XCH[3]])
            nc.scalar.dma_start(xs_sb[0:C1, XCH[1]:XCH[2]], xp_d.ap()[:, XCH[1]:XCH[2]])
            nc.sync.dma_start(xs_sb[0:C1, XCH[3]:XCH[4]], xp_d.ap()[:, XCH[3]:XCH[4]])
            nc.sync.dma_start(w2p_sb[:], w2_d.ap())
            nc.scalar.dma_start(b1_sb[:], b1_d.ap())
            nc.scalar.dma_start(b2_sb[:], b2_d.ap())
            # shift DMAs (dest chunk c reads source cols +1, all within the
            # same source chunk; boundaries offset by one column)
            nc.sync.dma_start(xs_sb[C1:128, XCH[0]:XCH[1] - 1],
                              xs_sb[0:C1, XCH[0] + 1:XCH[1]])
            nc.scalar.dma_start(xs_sb[C1:128, XCH[1] - 1:XCH[2] - 1],
                                xs_sb[0:C1, XCH[1]:XCH[2]])
            nc.gpsimd.dma_start(xs_sb[C1:128, XCH[2] - 1:XCH[3] - 1],
                                xs_sb[0:C1, XCH[2]:XCH[3]])
            nc.sync.dma_start(xs_sb[C1:128, XCH[3] - 1:XCH[4] - 1],
                              xs_sb[0:C1, XCH[3]:XCH[4]])

            # zero the padded contraction rows of q/k ONCE (both sides:
            # 0 * NaN-garbage would still poison the energy sums).  Whole
            # tile -- engine APs need 32-aligned partition bases; the
            # projections overwrite rows 0-7 later.
            nc.vector.memset(q_sb[:], 0.0)
            nc.vector.memset(k_sb[:], 0.0)

            nc.gpsimd.memset(ones_sb[:], 1.0)
            nc.gpsimd.memset(y_sb[C2:C2 + 1, :], 1.0)

            ys_v = ys_sb[:].rearrange("p (a b) -> p a b", b=WP)
            # zero only what conv2 actually reads as padding: rows 0/65
            # everywhere, pad cols 0/65 on the unshifted block (the shifted
            # blocks inherit correct pads from the chunked DMAs).
            nc.gpsimd.memset(ys_sb[:, 0:WP], 0.0)
            nc.gpsimd.memset(ys_sb[:, (HP - 1) * WP:NP], 0.0)
            nc.gpsimd.memset(ys_v[0:Cm, 1:HP - 1, 0:1], 0.0)
            nc.gpsimd.memset(ys_v[0:Cm, 1:HP - 1, WP - 1:WP], 0.0)

            vext_v = vext_sb[:].rearrange("p (c s k) -> p c s k", s=2, k=VP)
            nc.gpsimd.memset(vext_v[:, :, :, C2:], 0.0)
            nc.gpsimd.memset(vext_v[:, :, :, C2:C2 + 1], 1.0)

            xs_v = xs_sb[:].rearrange("p (a b) -> p a b", b=WP)
            y_v = y_sb[0:C2, :]
            y_rows = y_sb[0:C2, :].rearrange("p (a b) -> p a b", b=W)
            r_rows = r_sb[:].rearrange("p (a b) -> p a b", b=W)
            y65 = y_sb[:]

            # conv tiling: groups of RPT image rows; each tap streams one
            # CONTIGUOUS span of the padded image (garbage at the 2 pad
            # columns per row accumulates in psum and is skipped on
            # evacuation).
            conv_tiles = [(RPT * t, RPT) for t in range(H // RPT)]
            if H % RPT:
                conv_tiles.append((H - H % RPT, H % RPT))

            # conv1: 6 streamed passes/tile (the contraction-lower-bound):
            # 3x K=128 on xs=[x, x<<1] (taps (u,0)+(u,1)), 3x K=64 on the x
            # block at a +2 column offset (taps (u,2)).  Group 0 instead
            # runs 9x K=64 on the x block alone (taps (u,c) at +c column
            # offsets) so it isn't gated on the on-chip x<<1 shift DMA.
            def conv1_group(psA, tiles, nine=False):
                ps = psA.tile([128, WP * RPT], FP32, tag="mm")
                def mm(g, pass_i):
                    r0, nr = tiles[g]
                    length = WP * (nr - 1) + W
                    pslc = ps[Cm * g:Cm * (g + 1), 0:length]
                    tp = (0, Cm * g)
                    if nine:
                        u, c = pass_i // 3, pass_i % 3
                        lhs = (w1_sb[0:C1, Cm * u:Cm * (u + 1)] if c == 0 else
                               w1c_sb[:, Cm * u:Cm * (u + 1)] if c == 1 else
                               w1b_sb[:, Cm * u:Cm * (u + 1)])
                        s = (r0 + u) * WP + c
                        nc.tensor.matmul(
                            pslc, lhs, xs_sb[0:C1, s:s + length],
                            start=(pass_i == 0), stop=(pass_i == 8),
                            tile_position=tp,
                        )
                    elif pass_i < 3:
                        s = (r0 + pass_i) * WP
                        nc.tensor.matmul(
                            pslc, w1_sb[:, Cm * pass_i:Cm * (pass_i + 1)],
                            xs_sb[:, s:s + length],
                            start=(pass_i == 0), stop=False, tile_position=tp,
                        )
                    else:
                        u = pass_i - 3
                        s = (r0 + u) * WP + 2
                        nc.tensor.matmul(
                            pslc, w1b_sb[:, Cm * u:Cm * (u + 1)],
                            xs_sb[0:C1, s:s + length],
                            start=False, stop=(pass_i == 5), tile_position=tp,
                        )
                for pass_i in range(9 if nine else 6):
                    for g in range(len(tiles)):
                        mm(g, pass_i)
                ps_v = ps[:].rearrange("p (r w) -> p r w", w=WP)
                for g, (r0, nr) in enumerate(tiles):
                    nc.scalar.activation(
                        ys_v[0:Cm, 1 + r0:1 + r0 + nr, 1:1 + W],
                        ps_v[Cm * g:Cm * (g + 1), 0:nr, 0:W],
                        AF.Silu, bias=b1_sb[:, 0:1],
                    )

            # conv2: all 3 column taps on the partition axis (K=96, shifted
            # copies of y1 at rows 32-63 / 64-95): 3 passes instead of 9.
            def conv2_group(psA, tiles):
                ps = psA.tile([128, WP * RPT], FP32, tag="mm")
                for u in range(3):
                    for g, (r0, nr) in enumerate(tiles):
                        length = WP * (nr - 1) + W
                        s = (r0 + u) * WP
                        nc.tensor.matmul(
                            ps[C2 * g:C2 * (g + 1), 0:length],
                            w2_sb[:, C2 * u:C2 * (u + 1)],
                            ys_sb[:, s:s + length], start=(u == 0), stop=(u == 2),
                            tile_position=(0, C2 * g),
                        )
                ps_v = ps[:].rearrange("p (r w) -> p r w", w=WP)
                for g, (r0, nr) in enumerate(tiles):
                    nc.scalar.activation(
                        y_rows[:, r0:r0 + nr, :],
                        ps_v[C2 * g:C2 * (g + 1), 0:nr, 0:W],
                        AF.Silu, bias=b2_sb[:, 0:1],
                    )

            with tc.tile_pool(name="psA", bufs=6, space="PSUM") as psA:
                # conv1 with the y1 column-shift DMAs chunked in right behind
                # the tiles that produce their source rows.  First group is 2
                # tiles so it starts as soon as xs chunk 0 (cols 0:1056) lands.
                c1groups = [conv_tiles[0:2], conv_tiles[2:6], conv_tiles[6:10]]
                for gi, grp in enumerate(c1groups):
                    conv1_group(psA, grp, nine=(gi == 0))
                    for c0 in range(0, len(grp), 2):
                        sub = grp[c0:c0 + 2]
                        a = (1 + sub[0][0]) * WP
                        b = (1 + sub[-1][0] + sub[-1][1]) * WP
                        nc.sync.dma_start(ys_sb[Cm:2 * Cm, a - 1:b - 1],
                                          ys_sb[0:Cm, a:b])
                        nc.gpsimd.dma_start(ys_sb[2 * Cm:3 * Cm, a - 2:b - 2],
                                            ys_sb[0:Cm, a:b])

                # conv2 with projections (and the residual) interleaved; a
                # projection issues one tile AFTER its y span completes so the
                # in-order PE never parks on a SiLU that hasn't drained yet.
                k_done = q_done = vg_done = r_done = 0
                groups = [conv_tiles[i:i + 2] for i in range(0, len(conv_tiles), 2)]
                # the trailing k7/vg3 projections (rows 56-63) are NOT
                # flushed here: they are issued inside the attention loop
                # after pair 1 (from the psE pool) so the first energy/exp
                # chain isn't serialized behind them on the in-order PE --
                # they hide under exps 0-3 instead (the silu->exp
                # transition measured 5.3us).
                for grp in groups:
                    conv2_group(psA, grp)
                    rows = grp[0][0]
                    while q_done < min(4, rows // 8):
                        t0 = q_done
                        ps = psA.tile([C8, 512], FP32, tag="mm")
                        nc.tensor.matmul(ps[:], qw_sb[:],
                                         y65[:, 512 * t0:512 * (t0 + 1)],
                                         start=True, stop=True)
                        nc.vector.tensor_copy(q_sb[0:C8, 512 * t0:512 * (t0 + 1)],
                                              ps[:])
                        q_done += 1
                    while k_done < min(8, rows // 8):
                        t0 = k_done
                        ps = psA.tile([C8, 512], FP32, tag="mm")
                        nc.tensor.matmul(ps[:], kw_sb[:],
                                         y65[:, 512 * t0:512 * (t0 + 1)],
                                         start=True, stop=True)
                        nc.vector.tensor_copy(k_sb[0:C8, 512 * t0:512 * (t0 + 1)],
                                              ps[:])
                        k_done += 1
                    while vg_done < rows // 16:
                        g = vg_done
                        ps = psA.tile([128, 512], FP32, tag="mm")
                        for i in range(8):
                            j = 8 * g + i
                            nc.tensor.matmul(
                                ps[:, C2 * i:C2 * (i + 1)],
                                y65[:, 128 * j:128 * (j + 1)],
                                vw_sb[:],
                                start=True, stop=True,
                            )
                        nc.vector.tensor_copy(vext_v[:, 4 * g:4 * (g + 1), :, 0:C2],
                                              ps[:])
                        vg_done += 1
                    while r_done < min(2, rows // 16):
                        c = r_done
                        rs = slice(16 * c, 16 * (c + 1))
                        nc.vector.scalar_tensor_tensor(
                            r_rows[:, rs, :], y_rows[:, rs, :], 2.0,
                            xs_v[0:C1, 1 + 16 * c:1 + 16 * (c + 1), 1:1 + W],
                            ALU.mult, ALU.add,
                        )
                        r_done += 1

            # ---- attention, two sequential 1024-col halves ----
            with (
                tc.tile_pool(name="psE", bufs=3, space="PSUM") as psE,
                tc.tile_pool(name="psO", bufs=1, space="PSUM") as psO,
                tc.tile_pool(name="expp", bufs=4) as expp,
            ):
                for nh in range(2):
                    cs = NSPAN * nh
                    po = psO.tile([128, NSPAN], FP32, tag="po")
                    pending = []
                    dr_rem = []
                    for p in range(MCHUNKS // 2):
                        if nh == 0 and p == 2:
                            # deferred k tile 7 + v group 3 (psA is closed;
                            # borrow psE rotation slots).  k7 feeds pair 14+,
                            # vext 12-15 feeds DR pair 12+ -- plenty of lag.
                            psk = psE.tile([C8, 512], FP32, tag="pe")
                            nc.tensor.matmul(psk[:], kw_sb[:],
                                             y65[:, 512 * 7:512 * 8],
                                             start=True, stop=True)
                            nc.vector.tensor_copy(k_sb[0:C8, 512 * 7:512 * 8],
                                                  psk[:])
                            psv = psE.tile([128, 512], FP32, tag="pe")
                            for i in range(8):
                                jj = 24 + i
                                nc.tensor.matmul(
                                    psv[:, C2 * i:C2 * (i + 1)],
                                    y65[:, 128 * jj:128 * (jj + 1)], vw_sb[:],
                                    start=True, stop=True,
                                )
                            nc.vector.tensor_copy(vext_v[:, 12:16, :, 0:C2],
                                                  psv[:])
                        exv = expp.tile([128, 2 * NSPAN], FP8, tag="ex")
                        exv = exv[:].rearrange("q (s n) -> q s n", s=2)
                        for s in range(2):
                            i = 2 * p + s
                            pe = psE.tile([128, NSPAN], FP32, tag="pe")
                            for j in range(2):
                                # sandwich each lagged DR matmul between
                                # energy matmuls so its LDWEIGHTS hides
                                # under the neighbour's streaming
                                if dr_rem:
                                    _mm1(nc, po, vext_v, *dr_rem.pop(0))
                                nc.tensor.matmul(
                                    pe[:, 512 * j:512 * (j + 1)],
                                    k_sb[:, 128 * i:128 * (i + 1)],
                                    q_sb[:, cs + 512 * j:cs + 512 * (j + 1)],
                                    start=True, stop=True,
                                )
                            nc.scalar.activation(exv[:, s, :], pe[:], AF.Exp)
                        pending.append((exv, p))
                        if len(pending) > 3:
                            e0, p0 = pending.pop(0)
                            dr_rem += [(e0, p0, 0), (e0, p0, 1)]
                    for t in dr_rem:
                        _mm1(nc, po, vext_v, *t)
                    for e0, p0 in pending:
                        _mm1(nc, po, vext_v, e0, p0, 0)
                        _mm1(nc, po, vext_v, e0, p0, 1)

                    # ---- epilogue for this half, pipelined in 512-col blocks;
                    # all elementwise work on the (idle) DVE so ACT stays on
                    # the exp table for the next half's attention.
                    # 1/sumexp: bounce the [1,1024] denominator row through a
                    # [32,32] tile via DMA so the (per-column-costed) DVE
                    # reciprocal runs over 32 columns (0.36us) instead of 1024.
                    o_sb = per.tile([C2 + 1, NSPAN], FP32, tag="o_sb")
                    rec16 = per.tile([1, NSPAN], BF16, tag="rec16")
                    t1 = per.tile([C2, NSPAN], FP32, tag="t1")
                    zt0 = per.tile([16, 32], FP32, tag="zt0")
                    zr0 = per.tile([16, 32], FP32, tag="zr0")
                    zc0 = per.tile([16, 32], BF16, tag="zc0")
                    zt1 = per.tile([16, 32], FP32, tag="zt1")
                    zr1 = per.tile([16, 32], FP32, tag="zr1")
                    zc1 = per.tile([16, 32], BF16, tag="zc1")
                    zs = [(zt0, zr0, zc0), (zt1, zr1, zc1)]
                    # per-j denominator chains: the two SBUF bounce-DMA
                    # latencies pipeline instead of serializing.  Split
                    # [65,512] evacuations: rows 0-63 = numerator, row 64 =
                    # denominator (a separate [1,1024] copy costs a full
                    # 1024 per-column DVE pass on its own).
                    for j in range(2):
                        sl = slice(512 * j, 512 * (j + 1))
                        nc.vector.tensor_copy(o_sb[:, sl], po[0:C2 + 1, sl])
                        nc.sync.dma_start(zs[j][0][:], o_sb[C2:C2 + 1, sl])
                    for j in range(2):
                        sl = slice(512 * j, 512 * (j + 1))
                        nc.vector.reciprocal(zs[j][1][:], zs[j][0][:])
                        nc.vector.tensor_copy(zs[j][2][:], zs[j][1][:])
                        nc.sync.dma_start(rec16[:, sl], zs[j][2][:])
                    for j in range(2):
                        sl = slice(512 * j, 512 * (j + 1))
                        gl = slice(cs + 512 * j, cs + 512 * (j + 1))
                        nc.tensor.matmul(po[0:C2, sl], ones_sb[:], rec16[:, sl],
                                         start=True, stop=True)
                        nc.vector.tensor_mul(t1[:, sl], o_sb[0:C2, sl], po[0:C2, sl])
                        nc.vector.tensor_add(fin_sb[:, gl], t1[:, sl], r_sb[:, gl])
                        if nh == 0:
                            # sync only: the scalar queue is the ACT engine --
                            # a DMA trigger there would stall half 2's exps.
                            nc.sync.dma_start(out_d.ap()[:, gl], fin_sb[:, gl])
                        else:
                            ga, gb = cs + 512 * j, cs + 512 * j + 256
                            e0, e1 = (nc.sync, nc.gpsimd) if j == 0 else (
                                nc.scalar, nc.gpsimd)
                            e0.dma_start(out_d.ap()[:, ga:gb], fin_sb[:, ga:gb])
                            e1.dma_start(out_d.ap()[:, gb:gb + 256],
                                         fin_sb[:, gb:gb + 256])

    nc.compile()
    _build_cache["nc"] = nc
    return nc


def _mm1(nc, po, vext_v, exv, p, j):
    # DoubleRow fp8: contract 256 m-rows (chunk pair 2p, 2p+1) per pass.
    # po[m, n] += sum_s vext_{2p+s}[:, m]^T expT_{2p+s}[:, n]; row C2 = sum(exp)
    nc.tensor.matmul(
        po[0:VP, 512 * j:512 * (j + 1)],
        vext_v[:, p, :, :],
        exv[:, :, 512 * j:512 * (j + 1)],
        start=(p == 0), stop=(p == MCHUNKS // 2 - 1),
        perf_mode=mybir.MatmulPerfMode.DoubleRow,
    )


def _host_prep(inputs):
    f32 = np.float32
    x = np.asarray(inputs["x"], f32)
    s1 = np.asarray(inputs["bn1_g"], f32) / np.sqrt(np.asarray(inputs["bn1_v"], f32) + EPS)
    bb1 = np.asarray(inputs["bn1_b"], f32) - np.asarray(inputs["bn1_m"], f32) * s1
    w1 = np.asarray(inputs["cv1_w"], f32) * s1[:, None, None, None]
    s2 = np.asarray(inputs["bn2_g"], f32) / np.sqrt(np.asarray(inputs["bn2_v"], f32) + EPS)
    bb2 = np.asarray(inputs["bn2_b"], f32) - np.asarray(inputs["bn2_m"], f32) * s2
    w2 = np.asarray(inputs["cv2_w"], f32) * s2[:, None, None, None]
    gamma = f32(np.asarray(inputs["pam_gamma"], f32))

    bf = np.float16

    def aug(w, b):
        # [Co, C2] weight + [Co] bias -> [C2+1, Co] lhsT with bias row
        co = np.asarray(w, f32).shape[0]
        a = np.zeros((C2 + 1, co), f32)
        a[0:C2, :] = np.asarray(w, f32).T
        a[C2, :] = np.asarray(b, f32)
        return a.astype(bf)

    # w2pack: [96, 3*C2 | qwa(8) | kwa(8) | vwB(64)]
    w2pack = np.zeros((96, 3 * C2 + 2 * C8 + C2), bf)
    w2pack[0:C2 + 1, 3 * C2:3 * C2 + C8] = aug(inputs["q_w"], inputs["q_b"])
    w2pack[0:C2 + 1, 3 * C2 + C8:3 * C2 + 2 * C8] = aug(inputs["k_w"], inputs["k_b"])
    w2pack[0:C2 + 1, 3 * C2 + 2 * C8:] = aug(
        2.0 * gamma * np.asarray(inputs["v_w"], f32),
        2.0 * gamma * np.asarray(inputs["v_b"], f32))

    common = {
        "b1": np.ascontiguousarray(bb1[:, None]),
        "b2": np.ascontiguousarray(bb2[:, None]),
    }

    def packs(w1f, w2f):
        a = np.zeros((128, 9 * Cm), np.float32)
        s2p = np.zeros((96, 3 * C2), np.float32)
        for u in range(3):
            a[0:C1, Cm * u:Cm * (u + 1)] = w1f[:, :, u, 0].T
            a[C1:128, Cm * u:Cm * (u + 1)] = w1f[:, :, u, 1].T
            a[0:C1, 3 * Cm + Cm * u:3 * Cm + Cm * (u + 1)] = w1f[:, :, u, 2].T
            a[0:C1, 6 * Cm + Cm * u:6 * Cm + Cm * (u + 1)] = w1f[:, :, u, 1].T
            for j in range(3):
                s2p[Cm * j:Cm * (j + 1), C2 * u:C2 * (u + 1)] = w2f[:, :, u, j].T
        return a.astype(bf), s2p.astype(bf)

    wp = {0: packs(w1, w2), 1: packs(w1[:, :, ::-1, :], w2[:, :, ::-1, :])}

    in_maps = []
    for core in range(NCORES):
        b, fl = core // 2, core % 2
        xb = x[b] if fl == 0 else x[b][:, ::-1, :]
        xpad = np.zeros((C1, HP, WP), f32)
        xpad[:, 1:H + 1, 1:W + 1] = xb
        m = dict(common)
        m["xs"] = xpad.reshape(C1, NP).astype(np.float16)
        w1a, w2s = wp[fl]
        m["w1pack"] = w1a
        w2full = w2pack.copy()
        w2full[:, 0:3 * C2] = w2s
        m["w2pack"] = w2full
        in_maps.append(m)
    return in_maps


def _assemble(results):
    out = np.empty((B, C2, H, W), np.float32)
    for core in range(NCORES):
        b, fl = core // 2, core % 2
        o = results[core]["out"].reshape(C2, H // 2, W)
        if fl == 0:
            out[b, :, 0:H // 2, :] = o
        else:
            out[b, :, H // 2:H, :] = o[:, ::-1, :]
    return out


def _run(inputs, trace=False):
    nc = _build_program()
    in_maps = _host_prep(inputs)
    res = run_bass_kernel_spmd(nc, in_maps, core_ids=list(range(NCORES)), trace=trace)
    return _assemble(res.results), res


def kernel(**inputs):
    out, _ = _run(inputs)
    return out
